# revision 7
# baseline (speedup 1.0000x reference)
"""MLA multi-head latent attention kernel for 8 TRN2 NeuronCores.

Sharding: 8 cores = 2 batches (DP) x 4 head-groups of 4 heads (TP).
Each core computes its batch's shared LoRA down-projections plus its own
head-group's up-projections, attention, and partial o_proj; the host sums
the 4 per-group partial outputs per batch.

On-device everything is feature-major ([feature, T]) so no activation
transposes are needed anywhere:
  - host supplies x pre-transposed (xT [D, T]) and weights pre-sliced, with
    RMS-norm weights and the softmax scale folded into the up-projections
  - RMS-norm partition-dim reductions use an all-ones [128,128] stationary
    matmul (result replicated across partitions -> no broadcasts); the
    1/rms factor is constant along the contraction dim, so it is applied to
    matmul OUTPUTS at PSUM-eviction time (fused multiply) instead of inputs
  - attention computes scores transposed ([k, q]): softmax-sum over k is a
    ones-matmul; attn @ v uses v in T-major layout as the stationary operand
  - exp runs without max-subtraction (scores are small by construction)
  - matmuls run as float32r (full-rate fp32 relaxed-precision mode)
Output is produced feature-major (outT [D, T]); host transposes and sums.
"""
import math
import sys
from contextlib import ExitStack
from dataclasses import dataclass

sys.path.insert(0, '/opt/trn_rl_repo')
import numpy as np
import concourse.bass as bass
import concourse.bacc as bacc
import concourse.mybir as mybir
from concourse import tile
from concourse.bass_utils import run_bass_kernel_spmd

F32 = mybir.dt.float32
F32R = mybir.dt.float32r
AF = mybir.ActivationFunctionType


@dataclass
class Cfg:
    T: int = 2048
    D: int = 2048
    QL: int = 1536
    KVL: int = 512
    NHC: int = 4          # heads per core
    NOPE: int = 128
    ROPE: int = 64
    V: int = 128
    eps: float = 1e-6
    rope_base: float = 10000.0

    @property
    def NC(self):
        return min(512, self.T)

    @property
    def KD(self):
        return self.D // 128

    @property
    def MQ(self):
        return self.QL // 128

    @property
    def MKV(self):
        return self.KVL // 128

    @property
    def NT(self):
        return self.T // self.NC

    @property
    def TK(self):
        return self.T // 128

    @property
    def QROPE_CH(self):
        assert (self.NHC * self.ROPE) % 128 == 0
        return (self.NHC * self.ROPE) // 128

    @property
    def MASKW(self):
        return 2 * self.NC - 128

    @property
    def QH(self):
        return self.NOPE + self.ROPE

    @property
    def TH(self):         # phase-1 T-split groups
        return 2 if self.NT >= 2 else 1

    @property
    def NQM(self):        # q_b output 128-chunks
        return (self.NHC * self.QH) // 128


# full-scale problem constants (per harness contract)
B, T, D = 2, 2048, 2048
QL, KVL = 1536, 512
NHEADS, NOPE, ROPE, V = 16, 128, 64, 128
QH = NOPE + ROPE
NCORES = 8
GROUPS = 4
NHC = NHEADS // GROUPS
FULL = Cfg()


def _r(ap):
    return ap  # operands are float32r-typed


def build_nc(c: Cfg = FULL, num_devices: int = NCORES):
    nc = bacc.Bacc("TRN2", target_bir_lowering=False, debug=False,
                   num_devices=num_devices)
    W1C = c.QL + c.KVL + c.ROPE

    xT = nc.dram_tensor("xT", [c.D, c.T], F32R, kind="ExternalInput").ap()
    w1 = nc.dram_tensor("w1", [c.D, W1C], F32R, kind="ExternalInput").ap()
    qbw = nc.dram_tensor("qbw", [c.QL, c.NHC * c.QH], F32R, kind="ExternalInput").ap()
    kbw = nc.dram_tensor("kbw", [c.KVL, c.NHC * 128], F32R, kind="ExternalInput").ap()
    vbw = nc.dram_tensor("vbw", [c.KVL, c.NHC * c.V], F32R, kind="ExternalInput").ap()
    ow = nc.dram_tensor("ow", [c.NHC * c.V, c.D], F32R, kind="ExternalInput").ap()
    cos2 = nc.dram_tensor("cos2", [128, c.T], F32, kind="ExternalInput").ap()
    sin2 = nc.dram_tensor("sin2", [128, c.T], F32, kind="ExternalInput").ap()
    maskt = nc.dram_tensor("maskt", [128, c.MASKW], F32, kind="ExternalInput").ap()
    outT = nc.dram_tensor("outT", [c.D, c.T], F32, kind="ExternalOutput").ap()

    m1 = []
    off = 0
    while off < W1C:
        sz = min(128, W1C - off)
        m1.append((off, sz))
        off += sz
    m_order = list(range(c.MQ, len(m1))) + list(range(c.MQ))  # kv chunks first
    NTH = c.NT // c.TH                 # n-chunks per phase-1 T-group

    with tile.TileContext(nc) as tc, ExitStack() as top:
        dram = top.enter_context(tc.tile_pool(name="dram", bufs=1, space="DRAM"))
        scr_q = dram.tile([c.QL, c.T], F32R)
        scr_kv = dram.tile([c.KVL + c.ROPE, c.T], F32R)
        qT_d = dram.tile([c.NHC * c.QH, c.T], F32R)
        rkv_row = dram.tile([1, c.T], F32)

        const = top.enter_context(tc.tile_pool(name="const", bufs=1))
        ones_f = const.tile([128, 128], F32)
        nc.vector.memset(ones_f[:], 1.0)
        ones = const.tile([128, 128], F32R)
        nc.vector.tensor_copy(ones[:], ones_f[:])
        eps_sb = const.tile([128, 1], F32)
        nc.vector.memset(eps_sb[:], float(c.eps))
        rsq_q = const.tile([128, c.T], F32, tag="rsq_q")
        rsq_kv = const.tile([128, c.T], F32, tag="rsq_kv")

        kvc = top.enter_context(tc.tile_pool(name="kvc", bufs=1))  # "KV cache"
        knope = [kvc.tile([128, c.T], F32R, tag=f"kn{i}", name=f"kn{i}")
                 for i in range(c.NHC)]
        krope = kvc.tile([128, c.T], F32R, tag="krope")  # duplicated halves
        vsb = [kvc.tile([128, c.NHC * c.V], F32R, tag=f"v{i}", name=f"v{i}")
               for i in range(c.TK)]

        # ---------------- phase 1: x @ [q_a | kv_a | k_rope] ----------------
        with ExitStack() as p1:
            xt_pool = p1.enter_context(tc.tile_pool(name="xt", bufs=1))
            w1_pool = p1.enter_context(tc.tile_pool(name="w1", bufs=2))
            ev_pool = p1.enter_context(tc.tile_pool(name="p1ev", bufs=3))
            sq_pool = p1.enter_context(tc.tile_pool(name="p1sq", bufs=3))
            ps_pool = p1.enter_context(tc.tile_pool(name="p1ps", bufs=2, space="PSUM"))
            ssq_ps = p1.enter_context(tc.tile_pool(name="ssqps", bufs=5, space="PSUM"))

            THW = c.T // c.TH
            for th in range(c.TH):
                t0 = th * THW
                xt_sb = [xt_pool.tile([128, THW], F32R, tag=f"xt{k}", name=f"xt{k}")
                         for k in range(c.KD)]
                for k in range(c.KD):
                    nc.sync.dma_start(xt_sb[k][:],
                                      xT[k * 128:(k + 1) * 128, t0:t0 + THW])
                ssq_k = [None] * NTH
                ssq_q_t = [None] * NTH
                for mi in m_order:
                    m0, msz = m1[mi]
                    wt = w1_pool.tile([128, c.KD, 128], F32R, tag="w1t")
                    src = w1[:, m0:m0 + msz].rearrange("(k p) c -> p k c", p=128)
                    nc.sync.dma_start(wt[:, :, :msz], src)
                    is_q = mi < c.MQ
                    is_kr = m0 >= c.QL + c.KVL
                    for n in range(NTH):
                        lns = slice(n * c.NC, (n + 1) * c.NC)      # in xt tile
                        gns = slice(t0 + n * c.NC, t0 + (n + 1) * c.NC)
                        ps = ps_pool.tile([128, c.NC], F32, tag="ps")
                        for k in range(c.KD):
                            nc.tensor.matmul(ps[:msz, :], _r(wt[:, k, :msz]),
                                             _r(xt_sb[k][:, lns]),
                                             start=(k == 0), stop=(k == c.KD - 1))
                        ev = ev_pool.tile([128, c.NC], F32R, tag="ev")
                        nc.scalar.copy(ev[:msz, :], ps[:msz, :])
                        scr = scr_q if is_q else scr_kv
                        roff = m0 if is_q else m0 - c.QL
                        nc.sync.dma_start(scr[roff:roff + msz, gns], ev[:msz, :])
                        if not is_kr:
                            # ssq accumulation: square then ones-matmul
                            sq = sq_pool.tile([128, c.NC], F32R, tag="sq")
                            nc.scalar.square(sq[:msz, :], ps[:msz, :])
                            lst = ssq_q_t if is_q else ssq_k
                            if lst[n] is None:
                                lst[n] = ssq_ps.tile([128, c.NC], F32, tag="ssq",
                                                     name="ssq")
                            nmax = c.MQ if is_q else c.MKV
                            mloc = mi if is_q else mi - c.MQ
                            nc.tensor.matmul(lst[n][:], _r(ones[:msz, :]),
                                             _r(sq[:msz, :]),
                                             start=(mloc == 0),
                                             stop=(mloc == nmax - 1))

                    done_kv = (mi == c.MQ + c.MKV - 1)
                    done_q = (mi == c.MQ - 1)
                    if done_kv or done_q:
                        dim = c.KVL if done_kv else c.QL
                        tgt = rsq_kv if done_kv else rsq_q
                        lst = ssq_k if done_kv else ssq_q_t
                        for n in range(NTH):
                            gns = slice(t0 + n * c.NC, t0 + (n + 1) * c.NC)
                            nc.scalar.activation(tgt[:, gns], lst[n][:], AF.Sqrt,
                                                 bias=eps_sb[:], scale=1.0 / dim)
                        nc.vector.reciprocal(tgt[:, t0:t0 + THW],
                                             tgt[:, t0:t0 + THW])
                        if done_kv:
                            nc.sync.dma_start(rkv_row[0:1, t0:t0 + THW],
                                              rsq_kv[0:1, t0:t0 + THW])

        # ---------------- phase 2b: q_b (+ RoPE on q), staged to DRAM -------
        with ExitStack() as p2b:
            tb_pool = p2b.enter_context(tc.tile_pool(name="ropetb", bufs=1))
            cos_sb = tb_pool.tile([128, c.T], F32, tag="cos")
            sin_sb = tb_pool.tile([128, c.T], F32, tag="sin")
            nc.sync.dma_start(cos_sb[:], cos2[:])
            nc.sync.dma_start(sin_sb[:], sin2[:])

            qw_pool = p2b.enter_context(tc.tile_pool(name="qw", bufs=1))
            qbw_sb = [qw_pool.tile([128, c.NHC * c.QH], F32R, tag=f"qbw{k}",
                                   name=f"qbw{k}") for k in range(c.MQ)]
            for k in range(c.MQ):
                nc.sync.dma_start(qbw_sb[k][:], qbw[k * 128:(k + 1) * 128, :])

            xq_pool = p2b.enter_context(tc.tile_pool(name="xq", bufs=c.MQ + 2))
            ev_pool = p2b.enter_context(tc.tile_pool(name="p2ev", bufs=3))
            rt_pool = p2b.enter_context(tc.tile_pool(name="p2rt", bufs=2))
            ps2b = p2b.enter_context(tc.tile_pool(name="ps2b", bufs=2, space="PSUM"))

            def rope_cols(x_ap, rows, ns):
                # in-place rotate-half on [rows, NC] tile; tables sliced to ns
                tmp = rt_pool.tile([128, c.NC], F32R, tag="rtmp", name="rtmp")
                t1 = rt_pool.tile([128, c.NC], F32R, tag="rt1", name="rt1")
                for b0 in range(0, rows, 64):
                    nc.sync.dma_start(tmp[b0:b0 + 32, :], x_ap[b0 + 32:b0 + 64, :])
                    nc.sync.dma_start(tmp[b0 + 32:b0 + 64, :], x_ap[b0:b0 + 32, :])
                nc.vector.tensor_mul(tmp[:rows, :], tmp[:rows, :],
                                     sin_sb[:rows, ns])
                nc.vector.tensor_mul(t1[:rows, :], x_ap[:rows, :],
                                     cos_sb[:rows, ns])
                nc.vector.tensor_add(x_ap[:rows, :], t1[:rows, :], tmp[:rows, :])

            for n in range(c.NT):
                ns = slice(n * c.NC, (n + 1) * c.NC)
                xq = []
                for k in range(c.MQ):
                    t = xq_pool.tile([128, c.NC], F32R, tag="xq", name=f"xq{k}")
                    nc.sync.dma_start(t[:], scr_q[k * 128:(k + 1) * 128, ns])
                    xq.append(t)
                for m in range(c.NQM):
                    ps = ps2b.tile([128, c.NC], F32, tag="ps")
                    for k in range(c.MQ):
                        nc.tensor.matmul(ps[:], _r(qbw_sb[k][:, m * 128:(m + 1) * 128]),
                                         _r(xq[k][:]),
                                         start=(k == 0), stop=(k == c.MQ - 1))
                    ev = ev_pool.tile([128, c.NC], F32R, tag="ev")
                    nc.vector.tensor_mul(ev[:], ps[:], rsq_q[:, ns])
                    if m >= c.NHC:
                        rope_cols(ev[:], 128, ns)
                    nc.sync.dma_start(qT_d[m * 128:(m + 1) * 128, ns], ev[:])

            # k_rope: duplicate halves, rope in place (raw, not normed)
            nc.sync.dma_start(krope[0:64, :], scr_kv[c.KVL:c.KVL + c.ROPE, :])
            nc.sync.dma_start(krope[64:128, :], scr_kv[c.KVL:c.KVL + c.ROPE, :])
            for n in range(c.NT):
                ns = slice(n * c.NC, (n + 1) * c.NC)
                rope_cols(krope[:, ns], 128, ns)

        # ---------------- phase 2a: kv_b ----------------
        with ExitStack() as p2a:
            kvr_pool = p2a.enter_context(tc.tile_pool(name="kvr", bufs=1))
            kv_raw = [kvr_pool.tile([128, c.T], F32R, tag=f"kvr{k}", name=f"kvr{k}")
                      for k in range(c.MKV)]
            for k in range(c.MKV):
                nc.sync.dma_start(kv_raw[k][:], scr_kv[k * 128:(k + 1) * 128, :])

            kw_pool = p2a.enter_context(tc.tile_pool(name="kw", bufs=1))
            kbw_sb = [kw_pool.tile([128, c.NHC * 128], F32R, tag=f"kbw{k}",
                                   name=f"kbw{k}") for k in range(c.MKV)]
            vbw_sb = [kw_pool.tile([128, c.NHC * c.V], F32R, tag=f"vbw{k}",
                                   name=f"vbw{k}") for k in range(c.MKV)]
            for k in range(c.MKV):
                nc.sync.dma_start(kbw_sb[k][:], kbw[k * 128:(k + 1) * 128, :])
                nc.sync.dma_start(vbw_sb[k][:], vbw[k * 128:(k + 1) * 128, :])

            rc_pool = p2a.enter_context(tc.tile_pool(name="rcol", bufs=1))
            ps2 = p2a.enter_context(tc.tile_pool(name="ps2", bufs=2, space="PSUM"))

            for h in range(c.NHC):
                for n in range(c.NT):
                    ns = slice(n * c.NC, (n + 1) * c.NC)
                    ps = ps2.tile([128, c.NC], F32, tag="ps")
                    for k in range(c.MKV):
                        nc.tensor.matmul(ps[:], _r(kbw_sb[k][:, h * 128:(h + 1) * 128]),
                                         _r(kv_raw[k][:, ns]),
                                         start=(k == 0), stop=(k == c.MKV - 1))
                    nc.vector.tensor_mul(knope[h][:, ns], ps[:], rsq_kv[:, ns])
            for m in range(c.TK):
                ms = slice(m * 128, (m + 1) * 128)
                rcol = rc_pool.tile([128, 1], F32, tag=f"rc{m}", name=f"rc{m}")
                src = rkv_row[0:1, ms].rearrange("a (p o) -> (a p) o", p=128)
                nc.sync.dma_start(rcol[:], src)
                for nn0 in range(0, c.NHC * c.V, c.NC):
                    nn = slice(nn0, min(nn0 + c.NC, c.NHC * c.V))
                    nw = nn.stop - nn.start
                    ps = ps2.tile([128, c.NC], F32, tag="ps")
                    for k in range(c.MKV):
                        nc.tensor.matmul(ps[:, :nw], _r(kv_raw[k][:, ms]),
                                         _r(vbw_sb[k][:, nn]),
                                         start=(k == 0), stop=(k == c.MKV - 1))
                    nc.vector.tensor_scalar_mul(vsb[m][:, nn], ps[:, :nw], rcol[:])

        # ---------------- phase 3+4: attention then o_proj ----------------
        with ExitStack() as late:
            av_pool = late.enter_context(tc.tile_pool(name="avt", bufs=1))
            avt = [av_pool.tile([128, c.T], F32R, tag=f"av{i}", name=f"av{i}")
                   for i in range(c.NHC)]
            with ExitStack() as p3:
                mk_pool = p3.enter_context(tc.tile_pool(name="mask", bufs=1))
                mask_sb = mk_pool.tile([128, c.MASKW], F32)
                nc.sync.dma_start(mask_sb[:], maskt[:])
                qs_pool = p3.enter_context(tc.tile_pool(name="qs", bufs=2))
                s_ps = p3.enter_context(tc.tile_pool(name="sps", bufs=3, space="PSUM"))
                av_ps = p3.enter_context(tc.tile_pool(name="avps", bufs=2, space="PSUM"))
                sm_ps = p3.enter_context(tc.tile_pool(name="smps", bufs=2, space="PSUM"))
                e_pool = p3.enter_context(tc.tile_pool(name="e", bufs=4))
                rs_pool = p3.enter_context(tc.tile_pool(name="rs", bufs=2))

                for qn in range(c.NT):
                    qsl = slice(qn * c.NC, (qn + 1) * c.NC)
                    qtiles = []
                    for m in range(c.NQM):
                        t = qs_pool.tile([128, c.NC], F32R, tag=f"q{m}", name=f"q{m}")
                        nc.sync.dma_start(t[:], qT_d[m * 128:(m + 1) * 128, qsl])
                        qtiles.append(t)
                    nkj = ((qn + 1) * c.NC) // 128
                    for h in range(c.NHC):
                        q_nope = qtiles[h]
                        qr_t = qtiles[c.NHC + (h * 64) // 128]
                        qr_r0 = (h * 64) % 128
                        pav = av_ps.tile([128, c.NC], F32, tag="pav")
                        psm = sm_ps.tile([128, c.NC], F32, tag="psm")
                        for kj in range(nkj):
                            ks = slice(kj * 128, (kj + 1) * 128)
                            pss = s_ps.tile([128, c.NC], F32, tag="pss")
                            nc.tensor.matmul(pss[:], _r(knope[h][:, ks]),
                                             _r(q_nope[:]), start=True, stop=False)
                            nc.tensor.matmul(pss[:], _r(krope[qr_r0:qr_r0 + 64, ks]),
                                             _r(qr_t[qr_r0:qr_r0 + 64, :]),
                                             start=False, stop=True)
                            e = e_pool.tile([128, c.NC], F32R, tag="e")
                            off = kj * 128 - qn * c.NC
                            if off >= 0:  # diagonal tile: causal mask
                                msl = mask_sb[:, c.NC - 128 - off:2 * c.NC - 128 - off]
                                nc.vector.tensor_add(e[:], pss[:], msl)
                                nc.scalar.activation(e[:], e[:], AF.Exp)
                            else:
                                nc.scalar.activation(e[:], pss[:], AF.Exp)
                            first, last = (kj == 0), (kj == nkj - 1)
                            nc.tensor.matmul(pav[:], _r(vsb[kj][:, h * c.V:(h + 1) * c.V]),
                                             _r(e[:]), start=first, stop=last)
                            nc.tensor.matmul(psm[:], _r(ones[:]), _r(e[:]),
                                             start=first, stop=last)
                        rs = rs_pool.tile([128, c.NC], F32, tag="rs")
                        nc.vector.reciprocal(rs[:], psm[:])
                        nc.vector.tensor_mul(avt[h][:, qsl], pav[:], rs[:])

            with ExitStack() as p4:
                ow_pool = p4.enter_context(tc.tile_pool(name="ow", bufs=1))
                ow_sb = [ow_pool.tile([128, c.D], F32R, tag=f"ow{h}", name=f"ow{h}")
                         for h in range(c.NHC)]
                for h in range(c.NHC):
                    nc.sync.dma_start(ow_sb[h][:], ow[h * c.V:(h + 1) * c.V, :])
                ps4 = p4.enter_context(tc.tile_pool(name="ps4", bufs=2, space="PSUM"))
                ev4 = p4.enter_context(tc.tile_pool(name="ev4", bufs=3))
                for m in range(c.KD):
                    ms = slice(m * 128, (m + 1) * 128)
                    for n in range(c.NT):
                        ns = slice(n * c.NC, (n + 1) * c.NC)
                        ps = ps4.tile([128, c.NC], F32, tag="ps")
                        for h in range(c.NHC):
                            nc.tensor.matmul(ps[:], _r(ow_sb[h][:, ms]),
                                             _r(avt[h][:, ns]),
                                             start=(h == 0), stop=(h == c.NHC - 1))
                        ev = ev4.tile([128, c.NC], F32, tag="ev")
                        nc.scalar.copy(ev[:], ps[:])
                        nc.sync.dma_start(outT[ms, ns], ev[:])

    nc.compile()
    return nc


# ---------------- host-side prep ----------------
def make_tables(c: Cfg):
    j = np.arange(32, dtype=np.float64)
    invf = c.rope_base ** (-j / 32.0)
    pos = np.arange(c.T, dtype=np.float64)
    f = pos[:, None] * invf[None, :]
    cosT = np.cos(f).T.astype(np.float32)
    sinT = np.sin(f).T.astype(np.float32)
    cos64 = np.concatenate([cosT, cosT], 0)
    sin64 = np.concatenate([-sinT, sinT], 0)   # sign-folded rotate-half
    cos2 = np.concatenate([cos64, cos64], 0)
    sin2 = np.concatenate([sin64, sin64], 0)
    r = np.arange(128)[:, None]
    cc = np.arange(c.MASKW)[None, :]
    maskt = np.where(cc >= r + (c.NC - 128), 0.0, -1e30).astype(np.float32)
    return cos2, sin2, maskt


def make_core_inputs(c: Cfg, x, q_a_w, q_a_norm_w, q_b_w, kv_a_w, kv_norm_w,
                     kv_b_w, o_w, batch, heads):
    cos2, sin2, maskt = make_tables(c)
    scale = 1.0 / math.sqrt(c.QH)
    w1 = np.ascontiguousarray(np.concatenate([q_a_w, kv_a_w], axis=1))
    xT = np.ascontiguousarray(x[batch].T)
    qb = q_b_w.reshape(c.QL, -1, c.QH)
    qbw = np.concatenate([qb[:, h, :c.NOPE] for h in heads] +
                         [qb[:, h, c.NOPE:] for h in heads], axis=1)
    qbw = np.ascontiguousarray(qbw * q_a_norm_w[:, None] * scale)
    kvb = kv_b_w.reshape(c.KVL, -1, c.NOPE + c.V)
    kbw = np.concatenate([kvb[:, h, :c.NOPE] for h in heads], axis=1)
    kbw = np.ascontiguousarray(kbw * kv_norm_w[:, None])
    vbw = np.concatenate([kvb[:, h, c.NOPE:] for h in heads], axis=1)
    vbw = np.ascontiguousarray(vbw * kv_norm_w[:, None])
    o3 = o_w.reshape(-1, c.V, c.D)
    ows = np.ascontiguousarray(np.concatenate([o3[h] for h in heads], axis=0))
    return {'xT': xT, 'w1': w1, 'qbw': qbw, 'kbw': kbw, 'vbw': vbw, 'ow': ows,
            'cos2': cos2, 'sin2': sin2, 'maskt': maskt}


def prepare_in_maps(x, q_a_w, q_a_norm_w, q_b_w, kv_a_w, kv_norm_w, kv_b_w, o_w):
    args = [np.asarray(a, dtype=np.float32) for a in
            (x, q_a_w, q_a_norm_w, q_b_w, kv_a_w, kv_norm_w, kv_b_w, o_w)]
    in_maps = []
    for core in range(NCORES):
        b, g = core // GROUPS, core % GROUPS
        heads = list(range(g * NHC, (g + 1) * NHC))
        in_maps.append(make_core_inputs(FULL, *args, b, heads))
    return in_maps


def combine(results):
    out = np.zeros((B, T, D), dtype=np.float32)
    for core in range(NCORES):
        out[core // GROUPS] += results[core]['outT'].T
    return out


_NC_CACHE = None


def kernel(x, q_a_w, q_a_norm_w, q_b_w, kv_a_w, kv_norm_w, kv_b_w, o_w):
    global _NC_CACHE
    in_maps = prepare_in_maps(x, q_a_w, q_a_norm_w, q_b_w, kv_a_w, kv_norm_w,
                              kv_b_w, o_w)
    if _NC_CACHE is None:
        _NC_CACHE = build_nc()
    res = run_bass_kernel_spmd(_NC_CACHE, in_maps, core_ids=list(range(NCORES)))
    return combine(res.results)


# revision 8
# speedup vs baseline: 1.1609x; 1.1609x over previous
"""MLA multi-head latent attention kernel for 8 TRN2 NeuronCores.

Sharding: 8 cores = 2 batches (DP) x 4 head-groups of 4 heads (TP).
Each core computes its batch's shared LoRA down-projections plus its own
head-group's up-projections, attention, and partial o_proj; the host sums
the 4 per-group partial outputs per batch.

On-device everything is feature-major ([feature, T]) so no activation
transposes are needed anywhere:
  - host supplies x pre-transposed (xT [D, T]) and weights pre-sliced, with
    RMS-norm weights and the softmax scale folded into the up-projections
  - RMS-norm partition-dim reductions use an all-ones [128,128] stationary
    matmul (result replicated across partitions -> no broadcasts); the
    1/rms factor is constant along the contraction dim, so it is applied to
    matmul OUTPUTS at PSUM-eviction time (fused multiply) instead of inputs
  - attention computes scores transposed ([k, q]): softmax-sum over k is a
    ones-matmul; attn @ v uses v in T-major layout as the stationary operand
  - exp runs without max-subtraction (scores are small by construction)
  - matmuls run as float32r (full-rate fp32 relaxed-precision mode)
Output is produced feature-major (outT [D, T]); host transposes and sums.
"""
import math
import sys
from contextlib import ExitStack
from dataclasses import dataclass

sys.path.insert(0, '/opt/trn_rl_repo')
import numpy as np
import ml_dtypes
import concourse.bass as bass
import concourse.bacc as bacc
import concourse.mybir as mybir
from concourse import tile
from concourse.bass_utils import run_bass_kernel_spmd

F32 = mybir.dt.float32
F32R = mybir.dt.float32r
BF16 = mybir.dt.bfloat16
AF = mybir.ActivationFunctionType


@dataclass
class Cfg:
    T: int = 2048
    D: int = 2048
    QL: int = 1536
    KVL: int = 512
    NHC: int = 4          # heads per core
    NOPE: int = 128
    ROPE: int = 64
    V: int = 128
    eps: float = 1e-6
    rope_base: float = 10000.0

    @property
    def NC(self):
        return min(512, self.T)

    @property
    def KD(self):
        return self.D // 128

    @property
    def MQ(self):
        return self.QL // 128

    @property
    def MKV(self):
        return self.KVL // 128

    @property
    def NT(self):
        return self.T // self.NC

    @property
    def TK(self):
        return self.T // 128

    @property
    def QROPE_CH(self):
        assert (self.NHC * self.ROPE) % 128 == 0
        return (self.NHC * self.ROPE) // 128

    @property
    def MASKW(self):
        return 2 * self.NC - 128

    @property
    def QH(self):
        return self.NOPE + self.ROPE

    @property
    def TH(self):         # phase-1 T-split groups
        return 2 if self.NT >= 2 else 1

    @property
    def NQM(self):        # q_b output 128-chunks
        return (self.NHC * self.QH) // 128


# full-scale problem constants (per harness contract)
B, T, D = 2, 2048, 2048
QL, KVL = 1536, 512
NHEADS, NOPE, ROPE, V = 16, 128, 64, 128
QH = NOPE + ROPE
NCORES = 8
GROUPS = 4
NHC = NHEADS // GROUPS
FULL = Cfg()


def _r(ap):
    return ap  # operands are float32r-typed


def build_nc(c: Cfg = FULL, num_devices: int = NCORES):
    nc = bacc.Bacc("TRN2", target_bir_lowering=False, debug=False,
                   num_devices=num_devices)
    W1C = c.QL + c.KVL + c.ROPE

    xT = nc.dram_tensor("xT", [c.D, c.T], BF16, kind="ExternalInput").ap()
    w1 = nc.dram_tensor("w1", [c.D, W1C], BF16, kind="ExternalInput").ap()
    qbw = nc.dram_tensor("qbw", [c.QL, c.NHC * c.QH], BF16, kind="ExternalInput").ap()
    kbw = nc.dram_tensor("kbw", [c.KVL, c.NHC * 128], BF16, kind="ExternalInput").ap()
    vbw = nc.dram_tensor("vbw", [c.KVL, c.NHC * c.V], BF16, kind="ExternalInput").ap()
    ow = nc.dram_tensor("ow", [c.NHC * c.V, c.D], BF16, kind="ExternalInput").ap()
    cos2 = nc.dram_tensor("cos2", [128, c.T], BF16, kind="ExternalInput").ap()
    sin2 = nc.dram_tensor("sin2", [128, c.T], BF16, kind="ExternalInput").ap()
    maskt = nc.dram_tensor("maskt", [128, c.MASKW], F32, kind="ExternalInput").ap()
    outT = nc.dram_tensor("outT", [c.D, c.T], F32, kind="ExternalOutput").ap()

    m1 = []
    off = 0
    while off < W1C:
        sz = min(128, W1C - off)
        m1.append((off, sz))
        off += sz
    m_order = list(range(c.MQ, len(m1))) + list(range(c.MQ))  # kv chunks first
    NTH = c.NT // c.TH                 # n-chunks per phase-1 T-group

    with tile.TileContext(nc) as tc, ExitStack() as top:
        dram = top.enter_context(tc.tile_pool(name="dram", bufs=1, space="DRAM"))
        scr_q = dram.tile([c.QL, c.T], BF16)
        scr_kv = dram.tile([c.KVL + c.ROPE, c.T], BF16)
        qT_d = dram.tile([c.NHC * c.QH, c.T], BF16)
        rkv_row = dram.tile([1, c.T], F32)

        const = top.enter_context(tc.tile_pool(name="const", bufs=1))
        ones_f = const.tile([128, 128], F32)
        nc.vector.memset(ones_f[:], 1.0)
        ones = const.tile([128, 128], BF16)
        nc.vector.tensor_copy(ones[:], ones_f[:])
        eps_sb = const.tile([128, 1], F32)
        nc.vector.memset(eps_sb[:], float(c.eps))
        rsq_q = const.tile([128, c.T], F32, tag="rsq_q")
        rsq_kv = const.tile([128, c.T], F32, tag="rsq_kv")

        kvc = top.enter_context(tc.tile_pool(name="kvc", bufs=1))  # "KV cache"
        knope = [kvc.tile([128, c.T], BF16, tag=f"kn{i}", name=f"kn{i}")
                 for i in range(c.NHC)]
        krope = kvc.tile([128, c.T], BF16, tag="krope")  # duplicated halves
        vsb = [kvc.tile([128, c.NHC * c.V], BF16, tag=f"v{i}", name=f"v{i}")
               for i in range(c.TK)]

        # ---------------- phase 1: x @ [q_a | kv_a | k_rope] ----------------
        with ExitStack() as p1:
            xt_pool = p1.enter_context(tc.tile_pool(name="xt", bufs=1))
            w1_pool = p1.enter_context(tc.tile_pool(name="w1", bufs=2))
            ev_pool = p1.enter_context(tc.tile_pool(name="p1ev", bufs=3))
            sq_pool = p1.enter_context(tc.tile_pool(name="p1sq", bufs=3))
            ps_pool = p1.enter_context(tc.tile_pool(name="p1ps", bufs=2, space="PSUM"))
            ssq_ps = p1.enter_context(tc.tile_pool(name="ssqps", bufs=5, space="PSUM"))

            THW = c.T // c.TH
            for th in range(c.TH):
                t0 = th * THW
                xt_sb = [xt_pool.tile([128, THW], BF16, tag=f"xt{k}", name=f"xt{k}")
                         for k in range(c.KD)]
                for k in range(c.KD):
                    nc.sync.dma_start(xt_sb[k][:],
                                      xT[k * 128:(k + 1) * 128, t0:t0 + THW])
                ssq_k = [None] * NTH
                ssq_q_t = [None] * NTH
                for mi in m_order:
                    m0, msz = m1[mi]
                    wt = w1_pool.tile([128, c.KD, 128], BF16, tag="w1t")
                    src = w1[:, m0:m0 + msz].rearrange("(k p) c -> p k c", p=128)
                    nc.sync.dma_start(wt[:, :, :msz], src)
                    is_q = mi < c.MQ
                    is_kr = m0 >= c.QL + c.KVL
                    for n in range(NTH):
                        lns = slice(n * c.NC, (n + 1) * c.NC)      # in xt tile
                        gns = slice(t0 + n * c.NC, t0 + (n + 1) * c.NC)
                        ps = ps_pool.tile([128, c.NC], F32, tag="ps")
                        for k in range(c.KD):
                            nc.tensor.matmul(ps[:msz, :], _r(wt[:, k, :msz]),
                                             _r(xt_sb[k][:, lns]),
                                             start=(k == 0), stop=(k == c.KD - 1))
                        ev = ev_pool.tile([128, c.NC], BF16, tag="ev")
                        nc.scalar.copy(ev[:msz, :], ps[:msz, :])
                        scr = scr_q if is_q else scr_kv
                        roff = m0 if is_q else m0 - c.QL
                        nc.sync.dma_start(scr[roff:roff + msz, gns], ev[:msz, :])
                        if not is_kr:
                            # ssq accumulation: square then ones-matmul
                            sq = sq_pool.tile([128, c.NC], BF16, tag="sq")
                            nc.scalar.square(sq[:msz, :], ps[:msz, :])
                            lst = ssq_q_t if is_q else ssq_k
                            if lst[n] is None:
                                lst[n] = ssq_ps.tile([128, c.NC], F32, tag="ssq",
                                                     name="ssq")
                            nmax = c.MQ if is_q else c.MKV
                            mloc = mi if is_q else mi - c.MQ
                            nc.tensor.matmul(lst[n][:], _r(ones[:msz, :]),
                                             _r(sq[:msz, :]),
                                             start=(mloc == 0),
                                             stop=(mloc == nmax - 1))

                    done_kv = (mi == c.MQ + c.MKV - 1)
                    done_q = (mi == c.MQ - 1)
                    if done_kv or done_q:
                        dim = c.KVL if done_kv else c.QL
                        tgt = rsq_kv if done_kv else rsq_q
                        lst = ssq_k if done_kv else ssq_q_t
                        for n in range(NTH):
                            gns = slice(t0 + n * c.NC, t0 + (n + 1) * c.NC)
                            nc.scalar.activation(tgt[:, gns], lst[n][:], AF.Sqrt,
                                                 bias=eps_sb[:], scale=1.0 / dim)
                        nc.vector.reciprocal(tgt[:, t0:t0 + THW],
                                             tgt[:, t0:t0 + THW])
                        if done_kv:
                            nc.sync.dma_start(rkv_row[0:1, t0:t0 + THW],
                                              rsq_kv[0:1, t0:t0 + THW])

        # ---------------- phase 2b: q_b (+ RoPE on q), staged to DRAM -------
        with ExitStack() as p2b:
            tb_pool = p2b.enter_context(tc.tile_pool(name="ropetb", bufs=1))
            cos_sb = tb_pool.tile([128, c.T], BF16, tag="cos")
            sin_sb = tb_pool.tile([128, c.T], BF16, tag="sin")
            nc.sync.dma_start(cos_sb[:], cos2[:])
            nc.sync.dma_start(sin_sb[:], sin2[:])

            qw_pool = p2b.enter_context(tc.tile_pool(name="qw", bufs=1))
            qbw_sb = [qw_pool.tile([128, c.NHC * c.QH], BF16, tag=f"qbw{k}",
                                   name=f"qbw{k}") for k in range(c.MQ)]
            for k in range(c.MQ):
                nc.sync.dma_start(qbw_sb[k][:], qbw[k * 128:(k + 1) * 128, :])

            xq_pool = p2b.enter_context(tc.tile_pool(name="xq", bufs=c.MQ + 2))
            ev_pool = p2b.enter_context(tc.tile_pool(name="p2ev", bufs=3))
            rt_pool = p2b.enter_context(tc.tile_pool(name="p2rt", bufs=2))
            ps2b = p2b.enter_context(tc.tile_pool(name="ps2b", bufs=2, space="PSUM"))

            def rope_cols(x_ap, rows, ns):
                # in-place rotate-half on [rows, NC] tile; tables sliced to ns
                tmp = rt_pool.tile([128, c.NC], BF16, tag="rtmp", name="rtmp")
                t1 = rt_pool.tile([128, c.NC], BF16, tag="rt1", name="rt1")
                for b0 in range(0, rows, 64):
                    nc.sync.dma_start(tmp[b0:b0 + 32, :], x_ap[b0 + 32:b0 + 64, :])
                    nc.sync.dma_start(tmp[b0 + 32:b0 + 64, :], x_ap[b0:b0 + 32, :])
                nc.vector.tensor_mul(tmp[:rows, :], tmp[:rows, :],
                                     sin_sb[:rows, ns])
                nc.vector.tensor_mul(t1[:rows, :], x_ap[:rows, :],
                                     cos_sb[:rows, ns])
                nc.vector.tensor_add(x_ap[:rows, :], t1[:rows, :], tmp[:rows, :])

            for n in range(c.NT):
                ns = slice(n * c.NC, (n + 1) * c.NC)
                xq = []
                for k in range(c.MQ):
                    t = xq_pool.tile([128, c.NC], BF16, tag="xq", name=f"xq{k}")
                    nc.sync.dma_start(t[:], scr_q[k * 128:(k + 1) * 128, ns])
                    xq.append(t)
                for m in range(c.NQM):
                    ps = ps2b.tile([128, c.NC], F32, tag="ps")
                    for k in range(c.MQ):
                        nc.tensor.matmul(ps[:], _r(qbw_sb[k][:, m * 128:(m + 1) * 128]),
                                         _r(xq[k][:]),
                                         start=(k == 0), stop=(k == c.MQ - 1))
                    ev = ev_pool.tile([128, c.NC], BF16, tag="ev")
                    nc.vector.tensor_mul(ev[:], ps[:], rsq_q[:, ns])
                    if m >= c.NHC:
                        rope_cols(ev[:], 128, ns)
                    nc.sync.dma_start(qT_d[m * 128:(m + 1) * 128, ns], ev[:])

            # k_rope: duplicate halves, rope in place (raw, not normed)
            nc.sync.dma_start(krope[0:64, :], scr_kv[c.KVL:c.KVL + c.ROPE, :])
            nc.sync.dma_start(krope[64:128, :], scr_kv[c.KVL:c.KVL + c.ROPE, :])
            for n in range(c.NT):
                ns = slice(n * c.NC, (n + 1) * c.NC)
                rope_cols(krope[:, ns], 128, ns)

        # ---------------- phase 2a: kv_b ----------------
        with ExitStack() as p2a:
            kvr_pool = p2a.enter_context(tc.tile_pool(name="kvr", bufs=1))
            kv_raw = [kvr_pool.tile([128, c.T], BF16, tag=f"kvr{k}", name=f"kvr{k}")
                      for k in range(c.MKV)]
            for k in range(c.MKV):
                nc.sync.dma_start(kv_raw[k][:], scr_kv[k * 128:(k + 1) * 128, :])

            kw_pool = p2a.enter_context(tc.tile_pool(name="kw", bufs=1))
            kbw_sb = [kw_pool.tile([128, c.NHC * 128], BF16, tag=f"kbw{k}",
                                   name=f"kbw{k}") for k in range(c.MKV)]
            vbw_sb = [kw_pool.tile([128, c.NHC * c.V], BF16, tag=f"vbw{k}",
                                   name=f"vbw{k}") for k in range(c.MKV)]
            for k in range(c.MKV):
                nc.sync.dma_start(kbw_sb[k][:], kbw[k * 128:(k + 1) * 128, :])
                nc.sync.dma_start(vbw_sb[k][:], vbw[k * 128:(k + 1) * 128, :])

            rc_pool = p2a.enter_context(tc.tile_pool(name="rcol", bufs=1))
            ps2 = p2a.enter_context(tc.tile_pool(name="ps2", bufs=2, space="PSUM"))

            for h in range(c.NHC):
                for n in range(c.NT):
                    ns = slice(n * c.NC, (n + 1) * c.NC)
                    ps = ps2.tile([128, c.NC], F32, tag="ps")
                    for k in range(c.MKV):
                        nc.tensor.matmul(ps[:], _r(kbw_sb[k][:, h * 128:(h + 1) * 128]),
                                         _r(kv_raw[k][:, ns]),
                                         start=(k == 0), stop=(k == c.MKV - 1))
                    nc.vector.tensor_mul(knope[h][:, ns], ps[:], rsq_kv[:, ns])
            for m in range(c.TK):
                ms = slice(m * 128, (m + 1) * 128)
                rcol = rc_pool.tile([128, 1], F32, tag=f"rc{m}", name=f"rc{m}")
                src = rkv_row[0:1, ms].rearrange("a (p o) -> (a p) o", p=128)
                nc.sync.dma_start(rcol[:], src)
                for nn0 in range(0, c.NHC * c.V, c.NC):
                    nn = slice(nn0, min(nn0 + c.NC, c.NHC * c.V))
                    nw = nn.stop - nn.start
                    ps = ps2.tile([128, c.NC], F32, tag="ps")
                    for k in range(c.MKV):
                        nc.tensor.matmul(ps[:, :nw], _r(kv_raw[k][:, ms]),
                                         _r(vbw_sb[k][:, nn]),
                                         start=(k == 0), stop=(k == c.MKV - 1))
                    nc.vector.tensor_scalar_mul(vsb[m][:, nn], ps[:, :nw], rcol[:])

        # ---------------- phase 3+4: attention then o_proj ----------------
        with ExitStack() as late:
            av_pool = late.enter_context(tc.tile_pool(name="avt", bufs=1))
            avt = [av_pool.tile([128, c.T], BF16, tag=f"av{i}", name=f"av{i}")
                   for i in range(c.NHC)]
            with ExitStack() as p3:
                mk_pool = p3.enter_context(tc.tile_pool(name="mask", bufs=1))
                mask_sb = mk_pool.tile([128, c.MASKW], F32)
                nc.sync.dma_start(mask_sb[:], maskt[:])
                qs_pool = p3.enter_context(tc.tile_pool(name="qs", bufs=2))
                s_ps = p3.enter_context(tc.tile_pool(name="sps", bufs=3, space="PSUM"))
                av_ps = p3.enter_context(tc.tile_pool(name="avps", bufs=2, space="PSUM"))
                sm_ps = p3.enter_context(tc.tile_pool(name="smps", bufs=2, space="PSUM"))
                e_pool = p3.enter_context(tc.tile_pool(name="e", bufs=4))
                rs_pool = p3.enter_context(tc.tile_pool(name="rs", bufs=2))

                for qn in range(c.NT):
                    qsl = slice(qn * c.NC, (qn + 1) * c.NC)
                    qtiles = []
                    for m in range(c.NQM):
                        t = qs_pool.tile([128, c.NC], BF16, tag=f"q{m}", name=f"q{m}")
                        nc.sync.dma_start(t[:], qT_d[m * 128:(m + 1) * 128, qsl])
                        qtiles.append(t)
                    nkj = ((qn + 1) * c.NC) // 128
                    for h in range(c.NHC):
                        q_nope = qtiles[h]
                        qr_t = qtiles[c.NHC + (h * 64) // 128]
                        qr_r0 = (h * 64) % 128
                        pav = av_ps.tile([128, c.NC], F32, tag="pav")
                        psm = sm_ps.tile([128, c.NC], F32, tag="psm")
                        for kj in range(nkj):
                            ks = slice(kj * 128, (kj + 1) * 128)
                            pss = s_ps.tile([128, c.NC], F32, tag="pss")
                            nc.tensor.matmul(pss[:], _r(knope[h][:, ks]),
                                             _r(q_nope[:]), start=True, stop=False)
                            nc.tensor.matmul(pss[:], _r(krope[qr_r0:qr_r0 + 64, ks]),
                                             _r(qr_t[qr_r0:qr_r0 + 64, :]),
                                             start=False, stop=True)
                            e = e_pool.tile([128, c.NC], BF16, tag="e")
                            off = kj * 128 - qn * c.NC
                            if off >= 0:  # diagonal tile: causal mask
                                msl = mask_sb[:, c.NC - 128 - off:2 * c.NC - 128 - off]
                                nc.vector.tensor_add(e[:], pss[:], msl)
                                nc.scalar.activation(e[:], e[:], AF.Exp)
                            else:
                                nc.scalar.activation(e[:], pss[:], AF.Exp)
                            first, last = (kj == 0), (kj == nkj - 1)
                            nc.tensor.matmul(pav[:], _r(vsb[kj][:, h * c.V:(h + 1) * c.V]),
                                             _r(e[:]), start=first, stop=last)
                            nc.tensor.matmul(psm[:], _r(ones[:]), _r(e[:]),
                                             start=first, stop=last)
                        rs = rs_pool.tile([128, c.NC], F32, tag="rs")
                        nc.vector.reciprocal(rs[:], psm[:])
                        nc.vector.tensor_mul(avt[h][:, qsl], pav[:], rs[:])

            with ExitStack() as p4:
                ow_pool = p4.enter_context(tc.tile_pool(name="ow", bufs=1))
                ow_sb = [ow_pool.tile([128, c.D], BF16, tag=f"ow{h}", name=f"ow{h}")
                         for h in range(c.NHC)]
                for h in range(c.NHC):
                    nc.sync.dma_start(ow_sb[h][:], ow[h * c.V:(h + 1) * c.V, :])
                ps4 = p4.enter_context(tc.tile_pool(name="ps4", bufs=2, space="PSUM"))
                ev4 = p4.enter_context(tc.tile_pool(name="ev4", bufs=3))
                for m in range(c.KD):
                    ms = slice(m * 128, (m + 1) * 128)
                    for n in range(c.NT):
                        ns = slice(n * c.NC, (n + 1) * c.NC)
                        ps = ps4.tile([128, c.NC], F32, tag="ps")
                        for h in range(c.NHC):
                            nc.tensor.matmul(ps[:], _r(ow_sb[h][:, ms]),
                                             _r(avt[h][:, ns]),
                                             start=(h == 0), stop=(h == c.NHC - 1))
                        ev = ev4.tile([128, c.NC], F32, tag="ev")
                        nc.scalar.copy(ev[:], ps[:])
                        nc.sync.dma_start(outT[ms, ns], ev[:])

    nc.compile()
    return nc


# ---------------- host-side prep ----------------
def make_tables(c: Cfg):
    j = np.arange(32, dtype=np.float64)
    invf = c.rope_base ** (-j / 32.0)
    pos = np.arange(c.T, dtype=np.float64)
    f = pos[:, None] * invf[None, :]
    cosT = np.cos(f).T.astype(np.float32)
    sinT = np.sin(f).T.astype(np.float32)
    cos64 = np.concatenate([cosT, cosT], 0)
    sin64 = np.concatenate([-sinT, sinT], 0)   # sign-folded rotate-half
    cos2 = np.concatenate([cos64, cos64], 0).astype(ml_dtypes.bfloat16)
    sin2 = np.concatenate([sin64, sin64], 0).astype(ml_dtypes.bfloat16)
    r = np.arange(128)[:, None]
    cc = np.arange(c.MASKW)[None, :]
    maskt = np.where(cc >= r + (c.NC - 128), 0.0, -1e30).astype(np.float32)
    return cos2, sin2, maskt


def make_core_inputs(c: Cfg, x, q_a_w, q_a_norm_w, q_b_w, kv_a_w, kv_norm_w,
                     kv_b_w, o_w, batch, heads):
    cos2, sin2, maskt = make_tables(c)
    scale = 1.0 / math.sqrt(c.QH)
    w1 = np.ascontiguousarray(np.concatenate([q_a_w, kv_a_w], axis=1))
    xT = np.ascontiguousarray(x[batch].T)
    qb = q_b_w.reshape(c.QL, -1, c.QH)
    qbw = np.concatenate([qb[:, h, :c.NOPE] for h in heads] +
                         [qb[:, h, c.NOPE:] for h in heads], axis=1)
    qbw = np.ascontiguousarray(qbw * q_a_norm_w[:, None] * scale)
    kvb = kv_b_w.reshape(c.KVL, -1, c.NOPE + c.V)
    kbw = np.concatenate([kvb[:, h, :c.NOPE] for h in heads], axis=1)
    kbw = np.ascontiguousarray(kbw * kv_norm_w[:, None])
    vbw = np.concatenate([kvb[:, h, c.NOPE:] for h in heads], axis=1)
    vbw = np.ascontiguousarray(vbw * kv_norm_w[:, None])
    o3 = o_w.reshape(-1, c.V, c.D)
    ows = np.ascontiguousarray(np.concatenate([o3[h] for h in heads], axis=0))
    bf = ml_dtypes.bfloat16
    return {'xT': xT.astype(bf), 'w1': w1.astype(bf), 'qbw': qbw.astype(bf),
            'kbw': kbw.astype(bf), 'vbw': vbw.astype(bf), 'ow': ows.astype(bf),
            'cos2': cos2, 'sin2': sin2, 'maskt': maskt}


def prepare_in_maps(x, q_a_w, q_a_norm_w, q_b_w, kv_a_w, kv_norm_w, kv_b_w, o_w):
    args = [np.asarray(a, dtype=np.float32) for a in
            (x, q_a_w, q_a_norm_w, q_b_w, kv_a_w, kv_norm_w, kv_b_w, o_w)]
    in_maps = []
    for core in range(NCORES):
        b, g = core // GROUPS, core % GROUPS
        heads = list(range(g * NHC, (g + 1) * NHC))
        in_maps.append(make_core_inputs(FULL, *args, b, heads))
    return in_maps


def combine(results):
    out = np.zeros((B, T, D), dtype=np.float32)
    for core in range(NCORES):
        out[core // GROUPS] += results[core]['outT'].T
    return out


_NC_CACHE = None


def kernel(x, q_a_w, q_a_norm_w, q_b_w, kv_a_w, kv_norm_w, kv_b_w, o_w):
    global _NC_CACHE
    in_maps = prepare_in_maps(x, q_a_w, q_a_norm_w, q_b_w, kv_a_w, kv_norm_w,
                              kv_b_w, o_w)
    if _NC_CACHE is None:
        _NC_CACHE = build_nc()
    res = run_bass_kernel_spmd(_NC_CACHE, in_maps, core_ids=list(range(NCORES)))
    return combine(res.results)


# revision 9
# speedup vs baseline: 1.2332x; 1.0622x over previous
"""MLA multi-head latent attention kernel for 8 TRN2 NeuronCores.

Sharding: 8 cores = 2 batches (DP) x 4 head-groups of 4 heads (TP).
Each core computes its batch's shared LoRA down-projections plus its own
head-group's up-projections, attention, and partial o_proj; the host sums
the 4 per-group partial outputs per batch.

On-device everything is feature-major ([feature, T]) so no activation
transposes are needed anywhere:
  - host supplies x pre-transposed (xT [D, T]) and weights pre-sliced, with
    RMS-norm weights and the softmax scale folded into the up-projections
  - RMS-norm partition-dim reductions use an all-ones [128,128] stationary
    matmul (result replicated across partitions -> no broadcasts); the
    1/rms factor is constant along the contraction dim, so it is applied to
    matmul OUTPUTS at PSUM-eviction time (fused multiply) instead of inputs
  - attention computes scores transposed ([k, q]): softmax-sum over k is a
    ones-matmul; attn @ v uses v in T-major layout as the stationary operand
  - exp runs without max-subtraction (scores are small by construction)
  - matmuls run as float32r (full-rate fp32 relaxed-precision mode)
Output is produced feature-major (outT [D, T]); host transposes and sums.
"""
import math
import sys
from contextlib import ExitStack
from dataclasses import dataclass

sys.path.insert(0, '/opt/trn_rl_repo')
import numpy as np
import ml_dtypes
import concourse.bass as bass
import concourse.bacc as bacc
import concourse.mybir as mybir
from concourse import tile
from concourse.bass_utils import run_bass_kernel_spmd

F32 = mybir.dt.float32
F32R = mybir.dt.float32r
BF16 = mybir.dt.bfloat16
AF = mybir.ActivationFunctionType


@dataclass
class Cfg:
    T: int = 2048
    D: int = 2048
    QL: int = 1536
    KVL: int = 512
    NHC: int = 4          # heads per core
    NOPE: int = 128
    ROPE: int = 64
    V: int = 128
    eps: float = 1e-6
    rope_base: float = 10000.0

    @property
    def NC(self):
        return min(512, self.T)

    @property
    def KD(self):
        return self.D // 128

    @property
    def MQ(self):
        return self.QL // 128

    @property
    def MKV(self):
        return self.KVL // 128

    @property
    def NT(self):
        return self.T // self.NC

    @property
    def TK(self):
        return self.T // 128

    @property
    def QROPE_CH(self):
        assert (self.NHC * self.ROPE) % 128 == 0
        return (self.NHC * self.ROPE) // 128

    @property
    def MASKW(self):
        return 2 * self.NC - 128

    @property
    def QH(self):
        return self.NOPE + self.ROPE

    @property
    def TH(self):         # phase-1 T-split groups
        return 2 if self.NT >= 2 else 1

    @property
    def NQM(self):        # q_b output 128-chunks
        return (self.NHC * self.QH) // 128


# full-scale problem constants (per harness contract)
B, T, D = 2, 2048, 2048
QL, KVL = 1536, 512
NHEADS, NOPE, ROPE, V = 16, 128, 64, 128
QH = NOPE + ROPE
NCORES = 8
GROUPS = 4
NHC = NHEADS // GROUPS
FULL = Cfg()


def _r(ap):
    return ap  # operands are float32r-typed


def build_nc(c: Cfg = FULL, num_devices: int = NCORES):
    nc = bacc.Bacc("TRN2", target_bir_lowering=False, debug=False,
                   num_devices=num_devices)
    W1C = c.QL + c.KVL + c.ROPE

    xT = nc.dram_tensor("xT", [c.D, c.T], BF16, kind="ExternalInput").ap()
    w1 = nc.dram_tensor("w1", [c.D, W1C], BF16, kind="ExternalInput").ap()
    qbw = nc.dram_tensor("qbw", [c.QL, c.NHC * c.QH], BF16, kind="ExternalInput").ap()
    kbw = nc.dram_tensor("kbw", [c.KVL, c.NHC * 128], BF16, kind="ExternalInput").ap()
    vbw = nc.dram_tensor("vbw", [c.KVL, c.NHC * c.V], BF16, kind="ExternalInput").ap()
    ow = nc.dram_tensor("ow", [c.NHC * c.V, c.D], BF16, kind="ExternalInput").ap()
    cos2 = nc.dram_tensor("cos2", [128, c.T], BF16, kind="ExternalInput").ap()
    sin2 = nc.dram_tensor("sin2", [128, c.T], BF16, kind="ExternalInput").ap()
    maskt = nc.dram_tensor("maskt", [128, c.MASKW], F32, kind="ExternalInput").ap()
    outT = nc.dram_tensor("outT", [c.D, c.T], F32, kind="ExternalOutput").ap()

    m1 = []
    off = 0
    while off < W1C:
        sz = min(128, W1C - off)
        m1.append((off, sz))
        off += sz
    m_order = list(range(c.MQ, len(m1))) + list(range(c.MQ))  # kv chunks first
    NTH = c.NT // c.TH                 # n-chunks per phase-1 T-group

    with tile.TileContext(nc) as tc, ExitStack() as top:
        dram = top.enter_context(tc.tile_pool(name="dram", bufs=1, space="DRAM"))
        scr_q = dram.tile([c.QL, c.T], BF16)
        scr_kv = dram.tile([c.KVL + c.ROPE, c.T], BF16)
        qT_d = dram.tile([c.NHC * c.QH, c.T], BF16)
        rkv_row = dram.tile([1, c.T], F32)

        const = top.enter_context(tc.tile_pool(name="const", bufs=1))
        ones_f = const.tile([128, 128], F32)
        nc.vector.memset(ones_f[:], 1.0)
        ones = const.tile([128, 128], BF16)
        nc.vector.tensor_copy(ones[:], ones_f[:])
        eps_sb = const.tile([128, 1], F32)
        nc.vector.memset(eps_sb[:], float(c.eps))
        rsq_q = const.tile([128, c.T], F32, tag="rsq_q")
        rsq_kv = const.tile([128, c.T], F32, tag="rsq_kv")

        kvc = top.enter_context(tc.tile_pool(name="kvc", bufs=1))  # "KV cache"
        knope = [kvc.tile([128, c.T], BF16, tag=f"kn{i}", name=f"kn{i}")
                 for i in range(c.NHC)]
        krope = kvc.tile([128, c.T], BF16, tag="krope")  # duplicated halves
        vsb = [kvc.tile([128, c.NHC * c.V], BF16, tag=f"v{i}", name=f"v{i}")
               for i in range(c.TK)]

        # ---------------- phase 1: x @ [q_a | kv_a | k_rope] ----------------
        with ExitStack() as p1:
            xt_pool = p1.enter_context(tc.tile_pool(name="xt", bufs=2))
            w1_pool = p1.enter_context(tc.tile_pool(name="w1", bufs=2))
            ev_pool = p1.enter_context(tc.tile_pool(name="p1ev", bufs=3))
            sq_pool = p1.enter_context(tc.tile_pool(name="p1sq", bufs=3))
            ps_pool = p1.enter_context(tc.tile_pool(name="p1ps", bufs=2, space="PSUM"))
            ssq_ps = p1.enter_context(tc.tile_pool(name="ssqps", bufs=5, space="PSUM"))

            THW = c.T // c.TH
            for th in range(c.TH):
                t0 = th * THW
                xt_sb = [xt_pool.tile([128, THW], BF16, tag=f"xt{k}", name=f"xt{k}")
                         for k in range(c.KD)]
                for k in range(c.KD):
                    nc.sync.dma_start(xt_sb[k][:],
                                      xT[k * 128:(k + 1) * 128, t0:t0 + THW])
                ssq_k = [None] * NTH
                ssq_q_t = [None] * NTH
                for mi in m_order:
                    m0, msz = m1[mi]
                    wt = w1_pool.tile([128, c.KD, 128], BF16, tag="w1t")
                    src = w1[:, m0:m0 + msz].rearrange("(k p) c -> p k c", p=128)
                    nc.sync.dma_start(wt[:, :, :msz], src)
                    is_q = mi < c.MQ
                    is_kr = m0 >= c.QL + c.KVL
                    for n in range(NTH):
                        lns = slice(n * c.NC, (n + 1) * c.NC)      # in xt tile
                        gns = slice(t0 + n * c.NC, t0 + (n + 1) * c.NC)
                        ps = ps_pool.tile([128, c.NC], F32, tag="ps")
                        for k in range(c.KD):
                            nc.tensor.matmul(ps[:msz, :], _r(wt[:, k, :msz]),
                                             _r(xt_sb[k][:, lns]),
                                             start=(k == 0), stop=(k == c.KD - 1))
                        ev = ev_pool.tile([128, c.NC], BF16, tag="ev")
                        nc.scalar.copy(ev[:msz, :], ps[:msz, :])
                        scr = scr_q if is_q else scr_kv
                        roff = m0 if is_q else m0 - c.QL
                        nc.sync.dma_start(scr[roff:roff + msz, gns], ev[:msz, :])
                        if not is_kr:
                            # ssq accumulation: square then ones-matmul
                            sq = sq_pool.tile([128, c.NC], BF16, tag="sq")
                            nc.scalar.square(sq[:msz, :], ps[:msz, :])
                            lst = ssq_q_t if is_q else ssq_k
                            if lst[n] is None:
                                lst[n] = ssq_ps.tile([128, c.NC], F32, tag="ssq",
                                                     name="ssq")
                            nmax = c.MQ if is_q else c.MKV
                            mloc = mi if is_q else mi - c.MQ
                            nc.tensor.matmul(lst[n][:], _r(ones[:msz, :]),
                                             _r(sq[:msz, :]),
                                             start=(mloc == 0),
                                             stop=(mloc == nmax - 1))

                    done_kv = (mi == c.MQ + c.MKV - 1)
                    done_q = (mi == c.MQ - 1)
                    if done_kv or done_q:
                        dim = c.KVL if done_kv else c.QL
                        tgt = rsq_kv if done_kv else rsq_q
                        lst = ssq_k if done_kv else ssq_q_t
                        for n in range(NTH):
                            gns = slice(t0 + n * c.NC, t0 + (n + 1) * c.NC)
                            nc.scalar.activation(tgt[:, gns], lst[n][:], AF.Sqrt,
                                                 bias=eps_sb[:], scale=1.0 / dim)
                        nc.vector.reciprocal(tgt[:, t0:t0 + THW],
                                             tgt[:, t0:t0 + THW])
                        if done_kv:
                            nc.sync.dma_start(rkv_row[0:1, t0:t0 + THW],
                                              rsq_kv[0:1, t0:t0 + THW])

        # ---------------- phases 2b + 2a ----------------
        with ExitStack() as mid:
            # 2a operands hoisted: their DMAs depend only on early phase-1
            # kv scratch writes, so they land during 2b compute
            kvr_pool = mid.enter_context(tc.tile_pool(name="kvr", bufs=1))
            kv_raw = [kvr_pool.tile([128, c.T], BF16, tag=f"kvr{k}", name=f"kvr{k}")
                      for k in range(c.MKV)]
            for k in range(c.MKV):
                nc.sync.dma_start(kv_raw[k][:], scr_kv[k * 128:(k + 1) * 128, :])
            kw_pool = mid.enter_context(tc.tile_pool(name="kw", bufs=1))
            kbw_sb = [kw_pool.tile([128, c.NHC * 128], BF16, tag=f"kbw{k}",
                                   name=f"kbw{k}") for k in range(c.MKV)]
            vbw_sb = [kw_pool.tile([128, c.NHC * c.V], BF16, tag=f"vbw{k}",
                                   name=f"vbw{k}") for k in range(c.MKV)]
            for k in range(c.MKV):
                nc.sync.dma_start(kbw_sb[k][:], kbw[k * 128:(k + 1) * 128, :])
                nc.sync.dma_start(vbw_sb[k][:], vbw[k * 128:(k + 1) * 128, :])

            # ------------ phase 2b: q_b (+ RoPE on q), staged to DRAM -------
            p2b = mid.enter_context(ExitStack())
            tb_pool = p2b.enter_context(tc.tile_pool(name="ropetb", bufs=1))
            cos_sb = tb_pool.tile([128, c.T], BF16, tag="cos")
            sin_sb = tb_pool.tile([128, c.T], BF16, tag="sin")
            nc.sync.dma_start(cos_sb[:], cos2[:])
            nc.sync.dma_start(sin_sb[:], sin2[:])

            qw_pool = p2b.enter_context(tc.tile_pool(name="qw", bufs=1))
            qbw_sb = [qw_pool.tile([128, c.NHC * c.QH], BF16, tag=f"qbw{k}",
                                   name=f"qbw{k}") for k in range(c.MQ)]
            for k in range(c.MQ):
                nc.sync.dma_start(qbw_sb[k][:], qbw[k * 128:(k + 1) * 128, :])

            xq_pool = p2b.enter_context(tc.tile_pool(name="xq", bufs=c.MQ + 2))
            ev_pool = p2b.enter_context(tc.tile_pool(name="p2ev", bufs=3))
            rt_pool = p2b.enter_context(tc.tile_pool(name="p2rt", bufs=2))
            ps2b = p2b.enter_context(tc.tile_pool(name="ps2b", bufs=2, space="PSUM"))

            def rope_cols(x_ap, rows, ns):
                # in-place rotate-half on [rows, NC] tile; tables sliced to ns
                tmp = rt_pool.tile([128, c.NC], BF16, tag="rtmp", name="rtmp")
                t1 = rt_pool.tile([128, c.NC], BF16, tag="rt1", name="rt1")
                for b0 in range(0, rows, 64):
                    nc.sync.dma_start(tmp[b0:b0 + 32, :], x_ap[b0 + 32:b0 + 64, :])
                    nc.sync.dma_start(tmp[b0 + 32:b0 + 64, :], x_ap[b0:b0 + 32, :])
                nc.vector.tensor_mul(tmp[:rows, :], tmp[:rows, :],
                                     sin_sb[:rows, ns])
                nc.vector.tensor_mul(t1[:rows, :], x_ap[:rows, :],
                                     cos_sb[:rows, ns])
                nc.vector.tensor_add(x_ap[:rows, :], t1[:rows, :], tmp[:rows, :])

            for n in range(c.NT):
                ns = slice(n * c.NC, (n + 1) * c.NC)
                xq = []
                for k in range(c.MQ):
                    t = xq_pool.tile([128, c.NC], BF16, tag="xq", name=f"xq{k}")
                    nc.sync.dma_start(t[:], scr_q[k * 128:(k + 1) * 128, ns])
                    xq.append(t)
                for m in range(c.NQM):
                    ps = ps2b.tile([128, c.NC], F32, tag="ps")
                    for k in range(c.MQ):
                        nc.tensor.matmul(ps[:], _r(qbw_sb[k][:, m * 128:(m + 1) * 128]),
                                         _r(xq[k][:]),
                                         start=(k == 0), stop=(k == c.MQ - 1))
                    ev = ev_pool.tile([128, c.NC], BF16, tag="ev")
                    nc.vector.tensor_mul(ev[:], ps[:], rsq_q[:, ns])
                    if m >= c.NHC:
                        rope_cols(ev[:], 128, ns)
                    nc.sync.dma_start(qT_d[m * 128:(m + 1) * 128, ns], ev[:])

            # k_rope: duplicate halves, rope in place (raw, not normed)
            nc.sync.dma_start(krope[0:64, :], scr_kv[c.KVL:c.KVL + c.ROPE, :])
            nc.sync.dma_start(krope[64:128, :], scr_kv[c.KVL:c.KVL + c.ROPE, :])
            for n in range(c.NT):
                ns = slice(n * c.NC, (n + 1) * c.NC)
                rope_cols(krope[:, ns], 128, ns)

            p2b.close()

            # ------------ phase 2a: kv_b ------------
            p2a = mid.enter_context(ExitStack())
            rc_pool = p2a.enter_context(tc.tile_pool(name="rcol", bufs=1))
            ps2 = p2a.enter_context(tc.tile_pool(name="ps2", bufs=2, space="PSUM"))

            for h in range(c.NHC):
                for n in range(c.NT):
                    ns = slice(n * c.NC, (n + 1) * c.NC)
                    ps = ps2.tile([128, c.NC], F32, tag="ps")
                    for k in range(c.MKV):
                        nc.tensor.matmul(ps[:], _r(kbw_sb[k][:, h * 128:(h + 1) * 128]),
                                         _r(kv_raw[k][:, ns]),
                                         start=(k == 0), stop=(k == c.MKV - 1))
                    nc.vector.tensor_mul(knope[h][:, ns], ps[:], rsq_kv[:, ns])
            for m in range(c.TK):
                ms = slice(m * 128, (m + 1) * 128)
                rcol = rc_pool.tile([128, 1], F32, tag=f"rc{m}", name=f"rc{m}")
                src = rkv_row[0:1, ms].rearrange("a (p o) -> (a p) o", p=128)
                nc.sync.dma_start(rcol[:], src)
                for nn0 in range(0, c.NHC * c.V, c.NC):
                    nn = slice(nn0, min(nn0 + c.NC, c.NHC * c.V))
                    nw = nn.stop - nn.start
                    ps = ps2.tile([128, c.NC], F32, tag="ps")
                    for k in range(c.MKV):
                        nc.tensor.matmul(ps[:, :nw], _r(kv_raw[k][:, ms]),
                                         _r(vbw_sb[k][:, nn]),
                                         start=(k == 0), stop=(k == c.MKV - 1))
                    nc.vector.tensor_scalar_mul(vsb[m][:, nn], ps[:, :nw], rcol[:])

        # ---------------- phase 3+4: attention then o_proj ----------------
        with ExitStack() as late:
            av_pool = late.enter_context(tc.tile_pool(name="avt", bufs=1))
            avt = [av_pool.tile([128, c.T], BF16, tag=f"av{i}", name=f"av{i}")
                   for i in range(c.NHC)]
            ow_pool = late.enter_context(tc.tile_pool(name="ow", bufs=1))
            ow_sb = [ow_pool.tile([128, c.D], BF16, tag=f"ow{h}", name=f"ow{h}")
                     for h in range(c.NHC)]
            for h in range(c.NHC):
                nc.sync.dma_start(ow_sb[h][:], ow[h * c.V:(h + 1) * c.V, :])
            with ExitStack() as p3:
                mk_pool = p3.enter_context(tc.tile_pool(name="mask", bufs=1))
                mask_sb = mk_pool.tile([128, c.MASKW], F32)
                nc.sync.dma_start(mask_sb[:], maskt[:])
                qs_pool = p3.enter_context(tc.tile_pool(name="qs", bufs=2))
                s_ps = p3.enter_context(tc.tile_pool(name="sps", bufs=3, space="PSUM"))
                av_ps = p3.enter_context(tc.tile_pool(name="avps", bufs=2, space="PSUM"))
                sm_ps = p3.enter_context(tc.tile_pool(name="smps", bufs=2, space="PSUM"))
                e_pool = p3.enter_context(tc.tile_pool(name="e", bufs=4))
                rs_pool = p3.enter_context(tc.tile_pool(name="rs", bufs=2))

                for qn in range(c.NT):
                    qsl = slice(qn * c.NC, (qn + 1) * c.NC)
                    qtiles = []
                    for m in range(c.NQM):
                        t = qs_pool.tile([128, c.NC], BF16, tag=f"q{m}", name=f"q{m}")
                        nc.sync.dma_start(t[:], qT_d[m * 128:(m + 1) * 128, qsl])
                        qtiles.append(t)
                    nkj = ((qn + 1) * c.NC) // 128
                    for h in range(c.NHC):
                        q_nope = qtiles[h]
                        qr_t = qtiles[c.NHC + (h * 64) // 128]
                        qr_r0 = (h * 64) % 128
                        pav = av_ps.tile([128, c.NC], F32, tag="pav")
                        psm = sm_ps.tile([128, c.NC], F32, tag="psm")
                        for kj in range(nkj):
                            ks = slice(kj * 128, (kj + 1) * 128)
                            pss = s_ps.tile([128, c.NC], F32, tag="pss")
                            nc.tensor.matmul(pss[:], _r(knope[h][:, ks]),
                                             _r(q_nope[:]), start=True, stop=False)
                            nc.tensor.matmul(pss[:], _r(krope[qr_r0:qr_r0 + 64, ks]),
                                             _r(qr_t[qr_r0:qr_r0 + 64, :]),
                                             start=False, stop=True)
                            e = e_pool.tile([128, c.NC], BF16, tag="e")
                            off = kj * 128 - qn * c.NC
                            if off >= 0:  # diagonal tile: causal mask
                                msl = mask_sb[:, c.NC - 128 - off:2 * c.NC - 128 - off]
                                nc.vector.tensor_add(e[:], pss[:], msl)
                                nc.scalar.activation(e[:], e[:], AF.Exp)
                            else:
                                nc.scalar.activation(e[:], pss[:], AF.Exp)
                            first, last = (kj == 0), (kj == nkj - 1)
                            nc.tensor.matmul(pav[:], _r(vsb[kj][:, h * c.V:(h + 1) * c.V]),
                                             _r(e[:]), start=first, stop=last)
                            nc.tensor.matmul(psm[:], _r(ones[:]), _r(e[:]),
                                             start=first, stop=last)
                        rs = rs_pool.tile([128, c.NC], F32, tag="rs")
                        nc.vector.reciprocal(rs[:], psm[:])
                        nc.vector.tensor_mul(avt[h][:, qsl], pav[:], rs[:])

            with ExitStack() as p4:
                ps4 = p4.enter_context(tc.tile_pool(name="ps4", bufs=2, space="PSUM"))
                ev4 = p4.enter_context(tc.tile_pool(name="ev4", bufs=3))
                for m in range(c.KD):
                    ms = slice(m * 128, (m + 1) * 128)
                    for n in range(c.NT):
                        ns = slice(n * c.NC, (n + 1) * c.NC)
                        ps = ps4.tile([128, c.NC], F32, tag="ps")
                        for h in range(c.NHC):
                            nc.tensor.matmul(ps[:], _r(ow_sb[h][:, ms]),
                                             _r(avt[h][:, ns]),
                                             start=(h == 0), stop=(h == c.NHC - 1))
                        ev = ev4.tile([128, c.NC], F32, tag="ev")
                        nc.scalar.copy(ev[:], ps[:])
                        nc.sync.dma_start(outT[ms, ns], ev[:])

    nc.compile()
    return nc


# ---------------- host-side prep ----------------
def make_tables(c: Cfg):
    j = np.arange(32, dtype=np.float64)
    invf = c.rope_base ** (-j / 32.0)
    pos = np.arange(c.T, dtype=np.float64)
    f = pos[:, None] * invf[None, :]
    cosT = np.cos(f).T.astype(np.float32)
    sinT = np.sin(f).T.astype(np.float32)
    cos64 = np.concatenate([cosT, cosT], 0)
    sin64 = np.concatenate([-sinT, sinT], 0)   # sign-folded rotate-half
    cos2 = np.concatenate([cos64, cos64], 0).astype(ml_dtypes.bfloat16)
    sin2 = np.concatenate([sin64, sin64], 0).astype(ml_dtypes.bfloat16)
    r = np.arange(128)[:, None]
    cc = np.arange(c.MASKW)[None, :]
    maskt = np.where(cc >= r + (c.NC - 128), 0.0, -1e30).astype(np.float32)
    return cos2, sin2, maskt


def make_core_inputs(c: Cfg, x, q_a_w, q_a_norm_w, q_b_w, kv_a_w, kv_norm_w,
                     kv_b_w, o_w, batch, heads):
    cos2, sin2, maskt = make_tables(c)
    scale = 1.0 / math.sqrt(c.QH)
    w1 = np.ascontiguousarray(np.concatenate([q_a_w, kv_a_w], axis=1))
    xT = np.ascontiguousarray(x[batch].T)
    qb = q_b_w.reshape(c.QL, -1, c.QH)
    qbw = np.concatenate([qb[:, h, :c.NOPE] for h in heads] +
                         [qb[:, h, c.NOPE:] for h in heads], axis=1)
    qbw = np.ascontiguousarray(qbw * q_a_norm_w[:, None] * scale)
    kvb = kv_b_w.reshape(c.KVL, -1, c.NOPE + c.V)
    kbw = np.concatenate([kvb[:, h, :c.NOPE] for h in heads], axis=1)
    kbw = np.ascontiguousarray(kbw * kv_norm_w[:, None])
    vbw = np.concatenate([kvb[:, h, c.NOPE:] for h in heads], axis=1)
    vbw = np.ascontiguousarray(vbw * kv_norm_w[:, None])
    o3 = o_w.reshape(-1, c.V, c.D)
    ows = np.ascontiguousarray(np.concatenate([o3[h] for h in heads], axis=0))
    bf = ml_dtypes.bfloat16
    return {'xT': xT.astype(bf), 'w1': w1.astype(bf), 'qbw': qbw.astype(bf),
            'kbw': kbw.astype(bf), 'vbw': vbw.astype(bf), 'ow': ows.astype(bf),
            'cos2': cos2, 'sin2': sin2, 'maskt': maskt}


def prepare_in_maps(x, q_a_w, q_a_norm_w, q_b_w, kv_a_w, kv_norm_w, kv_b_w, o_w):
    args = [np.asarray(a, dtype=np.float32) for a in
            (x, q_a_w, q_a_norm_w, q_b_w, kv_a_w, kv_norm_w, kv_b_w, o_w)]
    in_maps = []
    for core in range(NCORES):
        b, g = core // GROUPS, core % GROUPS
        heads = list(range(g * NHC, (g + 1) * NHC))
        in_maps.append(make_core_inputs(FULL, *args, b, heads))
    return in_maps


def combine(results):
    out = np.zeros((B, T, D), dtype=np.float32)
    for core in range(NCORES):
        out[core // GROUPS] += results[core]['outT'].T
    return out


_NC_CACHE = None


def kernel(x, q_a_w, q_a_norm_w, q_b_w, kv_a_w, kv_norm_w, kv_b_w, o_w):
    global _NC_CACHE
    in_maps = prepare_in_maps(x, q_a_w, q_a_norm_w, q_b_w, kv_a_w, kv_norm_w,
                              kv_b_w, o_w)
    if _NC_CACHE is None:
        _NC_CACHE = build_nc()
    res = run_bass_kernel_spmd(_NC_CACHE, in_maps, core_ids=list(range(NCORES)))
    return combine(res.results)
